# revision 78
# baseline (speedup 1.0000x reference)
"""Trainium2 Bass/Tile kernel for MAB-style attention block (nn_MAB_channel_aware_force).

Reference computation (per batch b of 32):
  q = Q @ Wq + bq ; k = K @ Wk + bk ; v = K @ Wv + bv          # [512, 512]
  per head h (8 heads, dh=64):
    scores = qh @ kh^T / sqrt(512) ; A = softmax(scores)
    oh = qh + A @ vh
  O = LN0(concat(oh)) ; O = O + relu(O @ Wo + bo) ; out = LN1(O)

Sharding: data-parallel over batch across 8 NeuronCores (4 batches/core).

v5 structure per core (v4 + drain-packing / sums / engine rebalance):
  - transpose outputs pack two groups per PSUM tile ([128, 2, 512] bf16 =
    one bank): A/qnat/D/lnT drains halve in count and the mp ring's
    effective depth doubles, so PE transposes stop waiting on DVE.
  - both heads' A@V accumulate in one [128, 2, 512] f32 PSUM tile with a
    single [65, 2, 512] drain per pair.
  - softmax sums never round-trip DRAM: each head's sums row DMAs
    SBUF->SBUF into a [8, 512] stage tile; one tiny PE transpose per
    si-block + one reciprocal yields rn in natural [q, h] layout.
  - non-PSUM-gating work moves to the idle GpSimd engine: Q/K casts and
    VAUG copies for batches 1-3, Wo cast, mid-batch residual adds.
  - last-batch tail: stats interleave with per-si D groups, LN applies
    run on ACT (Identity w/ per-partition scale+bias), fc/p2/E2 chained
    per-si; batch-0 prologue orders engine queues so load casts lead
    weight casts (split per-dj / per-half for earlier availability).
"""

import numpy as np

import bass_rust as _bass_rust
import concourse.bass as bass
import concourse.mybir as mybir
import concourse.tile as tile
from concourse import bacc
from concourse.bass_utils import run_bass_kernel_spmd
from concourse.hw_specs import get_activation_tables
from concourse.masks import make_identity


class _BaccOneActTable(bacc.Bacc):
    """Bacc whose act-table pass is pinned to natural_log_exp_and_others.

    Every activation this kernel uses (exp, ln, copy, identity, relu)
    lives in the combined natural_log_exp_and_others set, so restricting
    the chooser to that set yields exactly one table load."""

    _ACT_SET = "natural_log_exp_and_others"

    def insert_act_table_loads(self):
        has_activation = any(
            isinstance(i, mybir.InstActivation)
            for b in self.main_func.blocks
            for i in b.instructions
        )
        if not has_activation:
            return
        tables = [
            (name, (fns if name == self._ACT_SET else set()))
            for name, fns in get_activation_tables(self.m.arch).items()
        ]
        _bass_rust.insert_act_table_loads(self, tables)

P = 128
S = 512          # sequence length (Sq == Sk)
D = 512          # model dim == DIM_Q == DIM_K == DIM_V
H = 8            # heads
DH = D // H      # 64
NB = 4           # batches per core
NCORES = 8
EPS = 1e-5
SC = 1.0 / float(np.sqrt(D))
F32 = mybir.dt.float32
BF16 = mybir.dt.bfloat16
AF = mybir.ActivationFunctionType
OP = mybir.AluOpType

NBLK = S // P    # 4 sequence blocks of 128
NDB = D // P     # 4 feature blocks of 128


def build_program(zero_bias: bool, unit_ln: bool):
    nc = _BaccOneActTable("TRN2", target_bir_lowering=False, debug=False)

    Qd = nc.declare_dram_parameter("Q", [NB, S, D], F32, isOutput=False)
    Kd = nc.declare_dram_parameter("K", [NB, S, D], F32, isOutput=False)
    Wd = {}
    for w in ("Wq", "Wk", "Wv", "Wo"):
        Wd[w] = nc.declare_dram_parameter(w, [D, D], F32, isOutput=False)
    Bd = {}
    for v in ("bq", "bk", "bv", "bo", "ln0_g", "ln0_b", "ln1_g", "ln1_b"):
        Bd[v] = nc.declare_dram_parameter(v, [D], F32, isOutput=False)
    Od = nc.declare_dram_parameter("out", [NB, S, D], F32, isOutput=True)

    with tile.TileContext(nc) as tc:
        _build(nc, tc, Qd, Kd, Wd, Bd, Od, zero_bias, unit_ln)
    nc.compile()
    return nc


def _build(nc, tc, Qd, Kd, Wd, Bd, Od, zero_bias, unit_ln):
    from contextlib import ExitStack

    ctx = ExitStack()
    with ctx:
        const = ctx.enter_context(tc.tile_pool(name="const", bufs=1))
        stage = ctx.enter_context(tc.tile_pool(name="stage", bufs=2))
        loadp = ctx.enter_context(tc.tile_pool(name="loadp", bufs=4))
        n16p = ctx.enter_context(tc.tile_pool(name="n16p", bufs=5))
        t16p = ctx.enter_context(tc.tile_pool(name="t16p", bufs=12))
        projp = ctx.enter_context(tc.tile_pool(name="projp", bufs=17))
        vaugp = ctx.enter_context(tc.tile_pool(name="vaugp", bufs=9))
        qnatp = ctx.enter_context(tc.tile_pool(name="qnatp", bufs=5))
        expp = ctx.enter_context(tc.tile_pool(name="expp", bufs=2))
        atp = ctx.enter_context(tc.tile_pool(name="atp", bufs=5))
        rnp = ctx.enter_context(tc.tile_pool(name="rnp", bufs=3))
        sump = ctx.enter_context(tc.tile_pool(name="sump", bufs=2))
        ohp = ctx.enter_context(tc.tile_pool(name="ohp", bufs=4))
        ln0p = ctx.enter_context(tc.tile_pool(name="ln0p", bufs=4))
        lntp = ctx.enter_context(tc.tile_pool(name="lntp", bufs=3))
        p2p = ctx.enter_context(tc.tile_pool(name="p2p", bufs=5))
        outp = ctx.enter_context(tc.tile_pool(name="outp", bufs=1))
        statp = ctx.enter_context(tc.tile_pool(name="statp", bufs=10))

        # PSUM (8 banks): scores pairs 2x[2 banks], attn-out pair 1x[2],
        # misc (proj/fc/transpose-pairs) 2x[1].
        scp = ctx.enter_context(tc.tile_pool(name="scp", bufs=2, space="PSUM"))
        pop = ctx.enter_context(tc.tile_pool(name="pop", bufs=1, space="PSUM"))
        mp = ctx.enter_context(tc.tile_pool(name="mp", bufs=2, space="PSUM"))

        # ---- one-time constants ----
        I128b = const.tile([P, P], BF16)
        make_identity(nc, I128b)
        I8f = const.tile([H, H], F32)
        make_identity(nc, I8f)
        epsT = const.tile([P, 1], F32)
        nc.vector.memset(epsT[:], EPS)

        W16 = {}
        WST = {}

        def emit_weight_dma(w):
            # split DMA in halves so the first dj blocks are usable before
            # the full matrix has landed
            st = loadp.tile([P, NDB, D], F32, tag="wld", name="wld", bufs=2)
            src = Wd[w].ap().rearrange("(o p) n -> p o n", p=P)
            for hh in range(2):
                nc.sync.dma_start(st[:, 2 * hh:2 * hh + 2, :],
                                  src[:, 2 * hh:2 * hh + 2, :])
            WST[w] = st

        def emit_weight_cast(w, eng):
            W16[w] = const.tile([P, NDB, D], BF16, tag=f"w16_{w}", name=f"w16_{w}")
            st = WST[w]
            for hh in range(2):
                sl = (slice(None), slice(2 * hh, 2 * hh + 2), slice(None))
                if eng == "act":
                    nc.scalar.activation(W16[w][sl], st[sl], AF.Copy)
                elif eng == "vec":
                    nc.vector.tensor_copy(W16[w][sl], st[sl])
                else:
                    nc.gpsimd.tensor_copy(W16[w][sl], st[sl])

        if not zero_bias:
            bqT = const.tile([P, NDB], F32, tag="bqT")
            nc.sync.dma_start(bqT[:], Bd["bq"].ap().rearrange("(o p) -> p o", p=P))
            bkT = const.tile([P, NDB], F32, tag="bkT")
            nc.sync.dma_start(bkT[:], Bd["bk"].ap().rearrange("(o p) -> p o", p=P))
            bc = {}
            for v in ("bv", "bo"):
                st = stage.tile([1, D], F32, tag="vstage")
                nc.sync.dma_start(st[:], Bd[v].ap()[None, :])
                bc[v] = const.tile([P, D], F32, tag=f"bc_{v}", name=f"bc_{v}")
                nc.gpsimd.partition_broadcast(bc[v][:], st[:])
            bv_bc, bo_bc = bc["bv"], bc["bo"]
        if not unit_ln:
            gbc = {}
            for v in ("ln0_g", "ln0_b", "ln1_g", "ln1_b"):
                st = stage.tile([1, D], F32, tag="vstage")
                nc.sync.dma_start(st[:], Bd[v].ap()[None, :])
                gbc[v] = const.tile([P, D], F32, tag=f"bc_{v}", name=f"bc_{v}")
                nc.gpsimd.partition_broadcast(gbc[v][:], st[:])

        # ---- per-batch state ----
        N16 = [{} for _ in range(NB)]       # name -> [128, NBLK, D] bf16
        T16S = [{} for _ in range(NB)]      # name -> [4 tiles d-major]
        PROJ = [{} for _ in range(NB)]      # "qT"/"kT" -> [4 tiles]
        VAUG = [None] * NB
        QNAT = [None] * NB                  # [4 AP views, natural q]
        AT = [[None] * H for _ in range(NB)]
        STG = [None] * NB                   # [H, S] bf16 sums stage
        RN = [None] * NB                    # [P, NBLK, H] f32 reciprocal sums
        OH = [None] * NB                    # [4 AP views, natural oh]
        LN0 = [None] * NB
        LNT = [None] * NB
        PRE2 = [None] * NB

        LDS = [{} for _ in range(NB)]
        RNPEND = [None] * NB

        def emit_load_dma(b, name):
            dram = Qd if name == "Q" else Kd
            ld = loadp.tile([P, NBLK, D], F32, tag="ld", name="ld")
            nc.sync.dma_start(
                ld[:], dram[b].rearrange("(si p) d -> p si d", p=P)
            )
            LDS[b][name] = ld

        def emit_load_cast(b, name):
            n16 = n16p.tile([P, NBLK, D], BF16, tag="n16", name="n16")
            if b == 0:
                # startup: per-dj casts on fast engines so A-groups start
                # as each feature block becomes available
                for dj in range(NDB):
                    sl = (slice(None), slice(None), slice(dj * P, (dj + 1) * P))
                    if name == "Q":
                        nc.scalar.activation(n16[sl], LDS[b][name][sl], AF.Copy)
                    else:
                        nc.vector.tensor_copy(n16[sl], LDS[b][name][sl])
            elif name == "Q":
                nc.scalar.activation(n16[:], LDS[b][name][:], AF.Copy)
            else:
                nc.vector.tensor_copy(n16[:], LDS[b][name][:])
            N16[b][name] = n16

        def ln_stats_si(src):
            mv = statp.tile([P, 2], F32, tag="mv1", name="mv1", bufs=6)
            st6 = statp.tile([P, 6], F32, tag="st6", name="st6")
            nc.vector.bn_stats(st6[:], src[:])
            nc.vector.bn_aggr(mv[:], st6[:])
            lnv = statp.tile([P, 1], F32, tag="lnv1", name="lnv1", bufs=6)
            nc.scalar.activation(lnv[:], mv[:, 1:2], AF.Ln, bias=epsT[:])
            istd = statp.tile([P, 1], F32, tag="istd1", name="istd1", bufs=6)
            nc.scalar.activation(istd[:], lnv[:], AF.Exp, scale=-0.5)
            return mv, istd

        def ln_apply_si(dst, src, mv, istd, g_bc, b_bc):
            if g_bc is None:
                nc.vector.tensor_scalar(
                    dst, src[:], mv[:, 0:1], istd[:], OP.subtract, OP.mult,
                )
            else:
                t = statp.tile([P, D], F32, tag="lntmp", name="lntmp")
                nc.vector.tensor_scalar(
                    t[:], src[:], mv[:, 0:1], istd[:], OP.subtract, OP.mult,
                )
                t2 = statp.tile([P, D], F32, tag="lntmp2", name="lntmp2")
                nc.vector.tensor_tensor(t2[:], t[:], g_bc[:], OP.mult)
                nc.vector.tensor_tensor(dst, t2[:], b_bc[:], OP.add)

        def ln_stats(srcs):
            """srcs: list of NBLK [128, 512] tiles -> (mv4, istd4)."""
            mv4 = statp.tile([P, NBLK, 2], F32, tag="mv4", name="mv4")
            for si in range(NBLK):
                st6 = statp.tile([P, 6], F32, tag="st6", name="st6")
                nc.vector.bn_stats(st6[:], srcs[si][:])
                nc.vector.bn_aggr(mv4[:, si, :], st6[:])
            lnv = statp.tile([P, NBLK], F32, tag="lnv", name="lnv")
            nc.scalar.activation(lnv[:], mv4[:, :, 1], AF.Ln, bias=epsT[:])
            istd4 = statp.tile([P, NBLK], F32, tag="istd4", name="istd4")
            nc.scalar.activation(istd4[:], lnv[:], AF.Exp, scale=-0.5)
            return mv4, istd4

        def ln_apply_into(dst, src, mv4, istd4, si, g_bc, b_bc):
            if g_bc is None:
                nc.vector.tensor_scalar(
                    dst, src[:], mv4[:, si, 0:1], istd4[:, si:si + 1],
                    OP.subtract, OP.mult,
                )
            else:
                t = statp.tile([P, D], F32, tag="lntmp", name="lntmp")
                nc.vector.tensor_scalar(
                    t[:], src[:], mv4[:, si, 0:1], istd4[:, si:si + 1],
                    OP.subtract, OP.mult,
                )
                t2 = statp.tile([P, D], F32, tag="lntmp2", name="lntmp2")
                nc.vector.tensor_tensor(t2[:], t[:], g_bc[:], OP.mult)
                nc.vector.tensor_tensor(dst, t2[:], b_bc[:], OP.add)

        def emit_A_group(b, g):
            # batch 0 (startup): one dj block per psum tile, DVE drain
            name, dj = ("Q", g) if g < NDB else ("K", g - NDB)
            n16 = N16[b][name]
            ps = mp.tile([P, S], BF16, tag="mp", name="psA")
            for si in range(NBLK):
                nc.tensor.transpose(
                    ps[:, si * P:(si + 1) * P],
                    n16[:, si, dj * P:(dj + 1) * P],
                    I128b[:],
                )
            t16 = t16p.tile([P, S], BF16, tag="t16", name="t16", bufs=8)
            nc.vector.tensor_copy(t16[:], ps[:])
            T16S[b].setdefault(name, []).append(t16)

        def emit_A_pair(b, name, dp):
            # steady state: two dj blocks share one [128, 2, 512] psum tile
            # and one drain
            n16 = N16[b][name]
            ps = mp.tile([P, 2, S], BF16, tag="mp", name="psA2")
            for u in range(2):
                dj = 2 * dp + u
                for si in range(NBLK):
                    nc.tensor.transpose(
                        ps[:, u, si * P:(si + 1) * P],
                        n16[:, si, dj * P:(dj + 1) * P],
                        I128b[:],
                    )
            t16 = t16p.tile([P, 2, S], BF16, tag="t16b", name="t16b", bufs=5)
            nc.vector.tensor_copy(t16[:], ps[:])
            lst = T16S[b].setdefault(name, [])
            lst.append(t16[:, 0, :])
            lst.append(t16[:, 1, :])

        PSB = [{} for _ in range(NB)]

        def emit_B_half(b, g, half):
            # B chains split in dj-halves: the open psum tile is held only
            # across ADJACENT filler pieces (no other mp alloc between), so
            # no single filler burst exceeds ~2 matmuls of PE time
            QT16 = T16S[b].get("Q")
            KT16 = T16S[b].get("K")
            djs = (0, 1) if half == 0 else (2, 3)
            if g < 8:  # qT (g 0-3) / kT (g 4-7)
                wname = "Wq" if g < NDB else "Wk"
                bT = None if zero_bias else (bqT if g < NDB else bkT)
                src = QT16 if g < NDB else KT16
                vi = g % NDB
                if half == 0:
                    ps = mp.tile([P, S], F32, tag="mp", name="psB")
                    PSB[b][g] = ps
                else:
                    ps = PSB[b].pop(g)
                for dj in djs:
                    nc.tensor.matmul(
                        ps[:],
                        W16[wname][:, dj, vi * P:(vi + 1) * P],
                        src[dj][:],
                        start=(dj == 0),
                        stop=(dj == NDB - 1),
                    )
                if half == 0:
                    return
                t = projp.tile([P, S], BF16, tag="projT", name="projT")
                # qT and kT drains on ACT (DVE is the busier engine)
                if bT is None:
                    nc.scalar.activation(t[:], ps[:], AF.Copy)
                else:
                    nc.scalar.activation(t[:], ps[:], AF.Identity,
                                         bias=bT[:, vi:vi + 1])
                PROJ[b].setdefault("qT" if g < NDB else "kT", []).append(t)
            else:      # v groups (g 8-11)
                si = g - 8
                if half == 0:
                    ps = mp.tile([P, S], F32, tag="mp", name="psV")
                    PSB[b][g] = ps
                else:
                    ps = PSB[b].pop(g)
                for dj in djs:
                    nc.tensor.matmul(
                        ps[:],
                        KT16[dj][:, si * P:(si + 1) * P],
                        W16["Wv"][:, dj, :],
                        start=(dj == 0),
                        stop=(dj == NDB - 1),
                    )
                if half == 0:
                    return
                if VAUG[b] is None:
                    VAUG[b] = []
                va = vaugp.tile([P, H, DH + 1], BF16, tag="vaug", name="vaug")
                nc.vector.memset(va[:, :, DH:DH + 1], 1.0)
                if zero_bias:
                    nc.vector.tensor_copy(
                        va[:, :, 0:DH], ps.rearrange("p (h d) -> p h d", h=H)
                    )
                else:
                    nc.vector.tensor_tensor(
                        va[:, :, 0:DH],
                        ps.rearrange("p (h d) -> p h d", h=H),
                        bv_bc.rearrange("p (h d) -> p h d", h=H),
                        OP.add,
                    )
                VAUG[b].append(va)

        def emit_B_group(b, g):
            emit_B_half(b, g, 0)
            emit_B_half(b, g, 1)

        def emit_qnat_pair(b, sp):
            # si blocks (2sp, 2sp+1) share one psum tile + one drain
            qT16 = PROJ[b]["qT"]
            ps = mp.tile([P, 2, S], BF16, tag="mp", name="psQn")
            for u in range(2):
                si = 2 * sp + u
                for vi in range(NDB):
                    nc.tensor.transpose(
                        ps[:, u, vi * P:(vi + 1) * P],
                        qT16[vi][:, si * P:(si + 1) * P],
                        I128b[:],
                    )
            if QNAT[b] is None:
                QNAT[b] = []
            qn = qnatp.tile([P, 2, S], BF16, tag="qnat", name="qnat")
            nc.vector.tensor_copy(qn[:], ps[:])
            QNAT[b].append(qn[:, 0, :])
            QNAT[b].append(qn[:, 1, :])

        def emit_C_pair(b, hp, filler=None):
            # heads (2hp, 2hp+1) share feature block hp; per ki both heads'
            # score matmuls -> one [128, 2, 512] PSUM tile, one exp per ki,
            # then both heads' A@V accumulate in one [128, 2, 512] psum
            # drained once per pair.  The PE stream is software-pipelined:
            # scores(ki+1) issue BEFORE A@V(ki) so the PE never sits queued
            # behind exp(ki); `filler` emits one interleave piece per
            # ki-step to keep every engine's queue fed.
            qT16, kT16 = PROJ[b]["qT"], PROJ[b]["kT"]
            if STG[b] is None:
                STG[b] = sump.tile([H, S], BF16, tag="stg", name="stg")
            vi = hp
            ea = expp.tile([P, 2, NBLK, S], BF16, tag="expA", name="expA")
            pos = pop.tile([P, 2, S], F32, tag="po", name="po")

            def emit_scores(ki):
                ps = scp.tile([P, 2, S], F32, tag="scp", name="scp")
                for u in range(2):
                    hof = u * DH
                    nc.tensor.matmul(
                        ps[:, u, :],
                        kT16[vi][hof:hof + DH, ki * P:(ki + 1) * P],
                        qT16[vi][hof:hof + DH, :],
                        start=True,
                        stop=True,
                    )
                return ps

            def emit_av(ki):
                for u in range(2):
                    h = 2 * hp + u
                    nc.tensor.matmul(
                        pos[0:DH + 1, u, :],
                        VAUG[b][ki][:, h, :],
                        ea[:, u, ki, :],
                        start=(ki == 0),
                        stop=(ki == NBLK - 1),
                    )

            for ki in range(NBLK):
                ps = emit_scores(ki)
                if ki >= 2:
                    emit_av(ki - 2)
                nc.scalar.activation(
                    ea[:, :, ki, :], ps[:], AF.Exp, scale=SC,
                )
                if filler is not None:
                    filler()
            emit_av(NBLK - 2)
            emit_av(NBLK - 1)
            at = atp.tile([DH + 1, 2, S], BF16, tag="at", name="at")
            if b == NB - 1 and hp == H // 2 - 1:
                # final pair: ACT is idle once its exps end, and the DVE
                # queue is the tail spine — drain there instead
                nc.scalar.activation(at[:], pos[0:DH + 1, :, :], AF.Copy)
            else:
                nc.vector.tensor_copy(at[:], pos[0:DH + 1, :, :])
            for u in range(2):
                h = 2 * hp + u
                # sums row -> stage tile (SBUF->SBUF DMA; no DRAM trip)
                nc.sync.dma_start(STG[b][h:h + 1, :], at[DH:DH + 1, u, :])
                AT[b][h] = at[:, u, :]

        PSD = [{} for _ in range(NB)]

        def emit_D_half(b, sp, half):
            # si blocks (2sp, 2sp+1): 16 transposes -> one [128, 2, 512]
            # psum -> one rn-mult drain -> two residual adds (gpsimd);
            # split in two adjacent filler pieces (8 transposes each)
            if half == 0:
                pa = mp.tile([P, 2, S], BF16, tag="mp", name="psD")
                PSD[b][sp] = pa
            else:
                pa = PSD[b].pop(sp)
            u = half
            si = 2 * sp + u
            for h in range(H):
                nc.tensor.transpose(
                    pa[:, u, h * DH:(h + 1) * DH],
                    AT[b][h][0:DH, si * P:(si + 1) * P],
                    I128b[0:DH, 0:DH],
                )
            if half == 0:
                return
            rn = RN[b]
            if OH[b] is None:
                OH[b] = []
            o = ohp.tile([P, 2, S], BF16, tag="oh", name="oh", bufs=3)
            nc.vector.tensor_tensor(
                o.rearrange("p u (h d) -> p u h d", h=H),
                pa.rearrange("p u (h d) -> p u h d", h=H),
                rn[:, 2 * sp:2 * sp + 2, :, None].to_broadcast((P, 2, H, DH)),
                OP.mult,
            )
            for u in range(2):
                si = 2 * sp + u
                nc.gpsimd.tensor_tensor(o[:, u, :], o[:, u, :],
                                        QNAT[b][si], OP.add)
                OH[b].append(o[:, u, :])

        def emit_rn(b):
            # stage [H, S] -> per-si PE transpose -> [P, si, H] psum -> 1/x
            ps = mp.tile([P, NBLK, H], BF16, tag="mp", name="psRn")
            for si in range(NBLK):
                nc.tensor.transpose(
                    ps[:, si, :],
                    STG[b][:, si * P:(si + 1) * P],
                    I128b[0:H, 0:H],
                )
            rn = rnp.tile([P, NBLK, H], F32, tag="rn", name="rn")
            nc.vector.reciprocal(rn[:], ps[:])
            RN[b] = rn

        def emit_D_pair(b, sp):
            emit_D_half(b, sp, 0)
            emit_D_half(b, sp, 1)

        def emit_D_group(b, si, last):
            # unpacked per-si variant (last batch: lower tail latency)
            rn = RN[b]
            pa = mp.tile([P, S], BF16, tag="mp", name="psD1")
            for h in range(H):
                nc.tensor.transpose(
                    pa[:, h * DH:(h + 1) * DH],
                    AT[b][h][0:DH, si * P:(si + 1) * P],
                    I128b[0:DH, 0:DH],
                )
            if OH[b] is None:
                OH[b] = []
            o = ohp.tile([P, S], BF16, tag="oh1", name="oh1", bufs=4)
            nc.vector.tensor_tensor(
                o.rearrange("p (h d) -> p h d", h=H),
                pa.rearrange("p (h d) -> p h d", h=H),
                rn[:, si, :, None].to_broadcast((P, H, DH)),
                OP.mult,
            )
            if last:
                nc.vector.tensor_tensor(o[:], o[:], QNAT[b][si], OP.add)
            else:
                nc.gpsimd.tensor_tensor(o[:], o[:], QNAT[b][si], OP.add)
            OH[b].append(o[:])

        MV4 = [None] * NB

        def emit_E_stats(b, si):
            if MV4[b] is None:
                MV4[b] = statp.tile([P, NBLK, 2], F32, tag="mv4", name="mv4")
            st6 = statp.tile([P, 6], F32, tag="st6", name="st6")
            nc.vector.bn_stats(st6[:], OH[b][si])
            nc.vector.bn_aggr(MV4[b][:, si, :], st6[:])

        def emit_E_fin(b):
            g0 = None if unit_ln else gbc["ln0_g"]
            b0 = None if unit_ln else gbc["ln0_b"]
            mv4 = MV4[b]
            lnv = statp.tile([P, NBLK], F32, tag="lnv", name="lnv")
            nc.scalar.activation(lnv[:], mv4[:, :, 1], AF.Ln, bias=epsT[:])
            istd4 = statp.tile([P, NBLK], F32, tag="istd4", name="istd4")
            nc.scalar.activation(istd4[:], lnv[:], AF.Exp, scale=-0.5)
            LN0[b] = []
            for si in range(NBLK):
                dst = ln0p.tile([P, D], BF16, tag="ln0", name="ln0")
                ln_apply_into(dst[:], OH[b][si], mv4, istd4, si, g0, b0)
                LN0[b].append(dst)

        def emit_F_lnT_pair(b, vp):
            # vi blocks (2vp, 2vp+1) share one psum tile + one drain
            ps = mp.tile([P, 2, S], BF16, tag="mp", name="psF")
            for u in range(2):
                vi = 2 * vp + u
                for si in range(NBLK):
                    nc.tensor.transpose(
                        ps[:, u, si * P:(si + 1) * P],
                        LN0[b][si][:, vi * P:(vi + 1) * P],
                        I128b[:],
                    )
            if LNT[b] is None:
                LNT[b] = []
            t = lntp.tile([P, 2, S], BF16, tag="lnT", name="lnT")
            nc.vector.tensor_copy(t[:], ps[:])
            LNT[b].append(t[:, 0, :])
            LNT[b].append(t[:, 1, :])

        PSF = [{} for _ in range(NB)]

        def emit_F_fc_half(b, si, half):
            if half == 0:
                ps = mp.tile([P, S], F32, tag="mp", name="psFc")
                PSF[b][si] = ps
            else:
                ps = PSF[b].pop(si)
            for dj in ((0, 1) if half == 0 else (2, 3)):
                nc.tensor.matmul(
                    ps[:],
                    LNT[b][dj][:, si * P:(si + 1) * P],
                    W16["Wo"][:, dj, :],
                    start=(dj == 0),
                    stop=(dj == NDB - 1),
                )
            if half == 0:
                return
            if PRE2[b] is None:
                PRE2[b] = []
            p2 = p2p.tile([P, D], BF16, tag="pre2", name="pre2")
            if zero_bias:
                # p2 = relu(fc) + ln0 fused: (ps max 0) + ln0
                nc.vector.scalar_tensor_tensor(
                    p2[:], ps[:], 0.0, LN0[b][si][:], OP.max, OP.add
                )
            else:
                tmp = statp.tile([P, D], F32, tag="fcb", name="fcb")
                nc.vector.tensor_tensor(tmp[:], ps[:], bo_bc[:], OP.add)
                rl = statp.tile([P, D], BF16, tag="relu", name="relu")
                nc.scalar.activation(rl[:], tmp[:], AF.Relu)
                nc.vector.tensor_tensor(p2[:], rl[:], LN0[b][si][:], OP.add)
            PRE2[b].append(p2)

        def emit_F_fc(b, si):
            emit_F_fc_half(b, si, 0)
            emit_F_fc_half(b, si, 1)

        MV4E = [None] * NB

        def emit_E2_stats(b, si):
            # per-si stats run as C-phase fillers so the batch-boundary
            # E2 leaves only Ln/Exp + applies + DMA
            if MV4E[b] is None:
                MV4E[b] = statp.tile([P, NBLK, 2], F32, tag="mv4e",
                                     name="mv4e", bufs=2)
            st6 = statp.tile([P, 6], F32, tag="st6", name="st6")
            nc.vector.bn_stats(st6[:], PRE2[b][si][:])
            nc.vector.bn_aggr(MV4E[b][:, si, :], st6[:])

        def emit_E2(b):
            g1 = None if unit_ln else gbc["ln1_g"]
            b1 = None if unit_ln else gbc["ln1_b"]
            if b == NB - 1:
                for si in range(NBLK):
                    mv, istd = ln_stats_si(PRE2[b][si])
                    of1 = outp.tile([P, D], F32, tag="outf1", name="outf1",
                                    bufs=2)
                    ln_apply_si(of1[:], PRE2[b][si], mv, istd, g1, b1)
                    nc.sync.dma_start(Od[b, si * P:(si + 1) * P, :], of1[:])
                return
            mv4b = MV4E[b]
            lnvb = statp.tile([P, NBLK], F32, tag="lnv", name="lnv")
            nc.scalar.activation(lnvb[:], mv4b[:, :, 1], AF.Ln, bias=epsT[:])
            istd4b = statp.tile([P, NBLK], F32, tag="istd4", name="istd4")
            nc.scalar.activation(istd4b[:], lnvb[:], AF.Exp, scale=-0.5)
            of = outp.tile([P, NBLK, D], F32, tag="outf", name="outf")
            if unit_ln:
                # E2 applies run in the ACT lull at the batch boundary; for
                # b < NB-2 alternate ACT/DVE, for NB-2 (right before the
                # last batch's DVE tail spine) run all four on ACT
                nm4b = statp.tile([P, NBLK], F32, tag="nm4", name="nm4")
                nc.vector.tensor_tensor(nm4b[:], mv4b[:, :, 0], istd4b[:],
                                        OP.mult)
                nc.vector.tensor_scalar(nm4b[:], nm4b[:], -1.0, None, OP.mult)
                for si in range(NBLK):
                    if b == NB - 2 or si % 2 == 0:
                        nc.scalar.activation(of[:, si, :], PRE2[b][si][:],
                                             AF.Identity,
                                             bias=nm4b[:, si:si + 1],
                                             scale=istd4b[:, si:si + 1])
                    else:
                        ln_apply_into(of[:, si, :], PRE2[b][si], mv4b,
                                      istd4b, si, g1, b1)
            else:
                for si in range(NBLK):
                    ln_apply_into(of[:, si, :], PRE2[b][si], mv4b, istd4b,
                                  si, g1, b1)
            nc.sync.dma_start(
                Od[b].rearrange("(si p) d -> p si d", p=P), of[:]
            )

        # ---- staged emission: 3 batches in flight ----
        def tail_pieces(b):
            th = []
            for sp in range(2):
                th.append(lambda sp=sp: emit_D_pair(b, sp))

            def emit_E(b=b):
                for si in range(NBLK):
                    emit_E_stats(b, si)
                emit_E_fin(b)

            th.append(emit_E)
            for vp in range(2):
                th.append(lambda vp=vp: emit_F_lnT_pair(b, vp))
            for si in range(NBLK):
                th.append(lambda si=si: emit_F_fc(b, si))
                th.append(lambda si=si: emit_E2_stats(b, si))
            return th  # 13 pieces; E2 finalize emitted separately

        def prep_pieces(nb):
            th = []
            for name in ("Q", "K"):
                for dp in range(2):
                    th.append(lambda name=name, dp=dp: emit_A_pair(nb, name, dp))
            for g in range(12):
                th.append(lambda g=g: emit_B_group(nb, g))
            return th  # 16 pieces; qnat emitted post-pairs

        # prologue: batch-0 path to first PE work.  Emission order is
        # engine-queue order, so per-engine the load casts lead the weight
        # casts and each stage's consumers directly follow its producers.
        emit_load_dma(0, "Q")
        emit_weight_dma("Wq")
        emit_load_dma(0, "K")
        emit_weight_dma("Wk")
        emit_load_cast(0, "Q")        # ACT per-dj
        for g in range(NDB):          # A-groups Q (t16 drains on DVE)
            emit_A_group(0, g)
        emit_weight_cast("Wq", "act")
        emit_load_cast(0, "K")        # DVE per-dj
        # Wv/Wo transfers deferred: Q/Wq/K/Wk keep full HBM bandwidth
        # through the startup-critical window
        emit_weight_dma("Wv")
        emit_weight_dma("Wo")
        for g in range(4):            # B qT chains (drains on ACT)
            emit_B_group(0, g)
        for g in range(NDB, 2 * NDB):  # A-groups K
            emit_A_group(0, g)
        emit_weight_cast("Wk", "vec")
        for g in range(4, 8):         # B kT chains
            emit_B_group(0, g)
        emit_weight_cast("Wv", "vec")
        for g in range(8, 12):        # B v chains -> VAUG
            emit_B_group(0, g)
        emit_weight_cast("Wo", "pool")
        for sp in range(2):
            emit_qnat_pair(0, sp)
        emit_load_dma(1, "Q")
        emit_load_dma(1, "K")

        for b in range(NB):
            nb = b + 1
            tails = tail_pieces(b - 1) if b > 0 else []
            preps = prep_pieces(nb) if nb < NB else []
            # A-pairs first so their PSUM drains lead the DVE queue (keeps
            # the mp ring moving), then D/E, then B chains, then lnT/fc
            inter = preps[:4] + tails[:3] + preps[4:] + tails[3:]
            if b == 0:
                inter = [lambda: emit_load_cast(1, "Q"),
                         lambda: emit_load_cast(1, "K")] + inter
            if b + 2 < NB:
                emit_load_dma(b + 2, "Q")
                emit_load_dma(b + 2, "K")
            # distribute interleave pieces one per ki-step (16 slots across
            # the 4 C-pairs) so mp-ring groups never run back-to-back
            state = {"fi": 0, "emitted": 0, "slot": 0}
            nslots = (H // 2) * NBLK

            def filler():
                state["slot"] += 1
                target = (len(inter) * state["slot"] + nslots - 1) // nslots
                while state["fi"] < len(inter) and state["emitted"] < target:
                    inter[state["fi"]]()
                    state["fi"] += 1
                    state["emitted"] += 1

            for hp in range(H // 2):
                emit_C_pair(b, hp, filler)
                if hp == H // 2 - 1:
                    emit_rn(b)
            while state["fi"] < len(inter):
                inter[state["fi"]]()
                state["fi"] += 1
            if b > 0:
                emit_E2(b - 1)
            if nb < NB:
                for sp in range(2):
                    emit_qnat_pair(nb, sp)
            if b + 2 < NB:
                emit_load_cast(b + 2, "Q")
                emit_load_cast(b + 2, "K")

        # epilogue: last batch tail; interleave stats with per-si D groups,
        # run LN applies on ACT, chain fc/p2/E2 per-si
        b = NB - 1
        if unit_ln:
            emit_D_group(b, 0, True)
            emit_D_group(b, 1, True)
            mv4 = statp.tile([P, NBLK, 2], F32, tag="mv4", name="mv4")

            def tail_stats(si):
                st6 = statp.tile([P, 6], F32, tag="st6", name="st6")
                nc.vector.bn_stats(st6[:], OH[b][si])
                nc.vector.bn_aggr(mv4[:, si, :], st6[:])

            tail_stats(0)
            emit_D_group(b, 2, True)
            tail_stats(1)
            emit_D_group(b, 3, True)
            tail_stats(2)
            tail_stats(3)
            lnv = statp.tile([P, NBLK], F32, tag="lnv", name="lnv")
            nc.scalar.activation(lnv[:], mv4[:, :, 1], AF.Ln, bias=epsT[:])
            istd4 = statp.tile([P, NBLK], F32, tag="istd4", name="istd4")
            nc.scalar.activation(istd4[:], lnv[:], AF.Exp, scale=-0.5)
            nm4 = statp.tile([P, NBLK], F32, tag="nm4", name="nm4")
            nc.vector.tensor_tensor(nm4[:], mv4[:, :, 0], istd4[:], OP.mult)
            nc.vector.tensor_scalar(nm4[:], nm4[:], -1.0, None, OP.mult)
            LN0[b] = []
            for si in range(NBLK):
                # alternate apply engines so the applies window halves and
                # each si's lnT transposes start sooner
                dst = ln0p.tile([P, D], BF16, tag="ln0", name="ln0")
                if si % 2 == 0:
                    nc.scalar.activation(dst[:], OH[b][si], AF.Identity,
                                         bias=nm4[:, si:si + 1],
                                         scale=istd4[:, si:si + 1])
                else:
                    nc.vector.tensor_scalar(
                        dst[:], OH[b][si], mv4[:, si, 0:1],
                        istd4[:, si:si + 1], OP.subtract, OP.mult,
                    )
                LN0[b].append(dst)
            # lnT transposes ordered si-major: each si's transposes chase
            # its LN apply, so the PE stays warm through the applies window
            # (both mp slots held; no other mp alloc until the drains)
            psF = [mp.tile([P, 2, S], BF16, tag="mp", name="psF")
                   for _ in range(2)]
            for si in range(NBLK):
                for vp in range(2):
                    for u in range(2):
                        vi = 2 * vp + u
                        nc.tensor.transpose(
                            psF[vp][:, u, si * P:(si + 1) * P],
                            LN0[b][si][:, vi * P:(vi + 1) * P],
                            I128b[:],
                        )
            LNT[b] = []
            for vp in range(2):
                t = lntp.tile([P, 2, S], BF16, tag="lnT", name="lnT")
                nc.vector.tensor_copy(t[:], psF[vp][:])
                LNT[b].append(t[:, 0, :])
                LNT[b].append(t[:, 1, :])
            for si in range(NBLK):
                emit_F_fc(b, si)
                # E2 per-si immediately after its fc/p2; applies alternate
                # ACT/DVE so consecutive si overlap
                mv, istd = ln_stats_si(PRE2[b][si])
                of1 = outp.tile([P, D], F32, tag="outf1", name="outf1",
                                bufs=2)
                if si % 2 == 0:
                    nm = statp.tile([P, 1], F32, tag="nm1", name="nm1",
                                    bufs=4)
                    nc.vector.tensor_scalar(
                        nm[:], mv[:, 0:1], istd[:], -1.0, OP.mult, OP.mult,
                    )
                    nc.scalar.activation(of1[:], PRE2[b][si][:], AF.Identity,
                                         bias=nm[:], scale=istd[:])
                else:
                    nc.vector.tensor_scalar(
                        of1[:], PRE2[b][si][:], mv[:, 0:1], istd[:],
                        OP.subtract, OP.mult,
                    )
                nc.sync.dma_start(Od[b, si * P:(si + 1) * P, :], of1[:])
        else:
            for th in tail_pieces(NB - 1):
                th()
            emit_E2(NB - 1)


_CACHE = {}


def _get_program(zero_bias: bool, unit_ln: bool):
    key = (zero_bias, unit_ln)
    if key not in _CACHE:
        _CACHE[key] = build_program(zero_bias, unit_ln)
    return _CACHE[key]


def _make_in_maps(inputs):
    Q = np.ascontiguousarray(inputs["Q"], dtype=np.float32)
    K = np.ascontiguousarray(inputs["K"], dtype=np.float32)
    shared = {
        name: np.ascontiguousarray(inputs[name], dtype=np.float32)
        for name in ("Wq", "Wk", "Wv", "Wo", "bq", "bk", "bv", "bo",
                     "ln0_g", "ln0_b", "ln1_g", "ln1_b")
    }
    in_maps = []
    for c in range(NCORES):
        m = dict(shared)
        m["Q"] = Q[c * NB:(c + 1) * NB]
        m["K"] = K[c * NB:(c + 1) * NB]
        in_maps.append(m)
    return in_maps


def run(inputs, trace=False):
    zero_bias = all(
        not np.any(inputs[v]) for v in ("bq", "bk", "bv", "bo")
    )
    unit_ln = (
        np.all(inputs["ln0_g"] == 1.0) and np.all(inputs["ln1_g"] == 1.0)
        and not np.any(inputs["ln0_b"]) and not np.any(inputs["ln1_b"])
    )
    nc = _get_program(zero_bias, unit_ln)
    res = run_bass_kernel_spmd(
        nc, _make_in_maps(inputs), core_ids=list(range(NCORES)), trace=trace
    )
    out = np.concatenate([res.results[c]["out"] for c in range(NCORES)], axis=0)
    return out, res


def kernel(**inputs):
    B, Sq, Dq = inputs["Q"].shape
    assert (B, Sq, Dq) == (NB * NCORES, S, D), (B, Sq, Dq)
    out, _ = run(inputs, trace=False)
    return out


# revision 79
# speedup vs baseline: 1.0070x; 1.0070x over previous
"""Trainium2 Bass/Tile kernel for MAB-style attention block (nn_MAB_channel_aware_force).

Reference computation (per batch b of 32):
  q = Q @ Wq + bq ; k = K @ Wk + bk ; v = K @ Wv + bv          # [512, 512]
  per head h (8 heads, dh=64):
    scores = qh @ kh^T / sqrt(512) ; A = softmax(scores)
    oh = qh + A @ vh
  O = LN0(concat(oh)) ; O = O + relu(O @ Wo + bo) ; out = LN1(O)

Sharding: data-parallel over batch across 8 NeuronCores (4 batches/core).

v5 structure per core (v4 + drain-packing / sums / engine rebalance):
  - transpose outputs pack two groups per PSUM tile ([128, 2, 512] bf16 =
    one bank): A/qnat/D/lnT drains halve in count and the mp ring's
    effective depth doubles, so PE transposes stop waiting on DVE.
  - both heads' A@V accumulate in one [128, 2, 512] f32 PSUM tile with a
    single [65, 2, 512] drain per pair.
  - softmax sums never round-trip DRAM: each head's sums row DMAs
    SBUF->SBUF into a [8, 512] stage tile; one tiny PE transpose per
    si-block + one reciprocal yields rn in natural [q, h] layout.
  - non-PSUM-gating work moves to the idle GpSimd engine: Q/K casts and
    VAUG copies for batches 1-3, Wo cast, mid-batch residual adds.
  - last-batch tail: stats interleave with per-si D groups, LN applies
    run on ACT (Identity w/ per-partition scale+bias), fc/p2/E2 chained
    per-si; batch-0 prologue orders engine queues so load casts lead
    weight casts (split per-dj / per-half for earlier availability).
"""

import numpy as np

import bass_rust as _bass_rust
import concourse.bass as bass
import concourse.mybir as mybir
import concourse.tile as tile
from concourse import bacc
from concourse.bass_utils import run_bass_kernel_spmd
from concourse.hw_specs import get_activation_tables
from concourse.masks import make_identity


class _BaccOneActTable(bacc.Bacc):
    """Bacc whose act-table pass is pinned to natural_log_exp_and_others.

    Every activation this kernel uses (exp, ln, copy, identity, relu)
    lives in the combined natural_log_exp_and_others set, so restricting
    the chooser to that set yields exactly one table load."""

    _ACT_SET = "natural_log_exp_and_others"

    def insert_act_table_loads(self):
        has_activation = any(
            isinstance(i, mybir.InstActivation)
            for b in self.main_func.blocks
            for i in b.instructions
        )
        if not has_activation:
            return
        tables = [
            (name, (fns if name == self._ACT_SET else set()))
            for name, fns in get_activation_tables(self.m.arch).items()
        ]
        _bass_rust.insert_act_table_loads(self, tables)

P = 128
S = 512          # sequence length (Sq == Sk)
D = 512          # model dim == DIM_Q == DIM_K == DIM_V
H = 8            # heads
DH = D // H      # 64
NB = 4           # batches per core
NCORES = 8
EPS = 1e-5
SC = 1.0 / float(np.sqrt(D))
F32 = mybir.dt.float32
BF16 = mybir.dt.bfloat16
AF = mybir.ActivationFunctionType
OP = mybir.AluOpType

NBLK = S // P    # 4 sequence blocks of 128
NDB = D // P     # 4 feature blocks of 128


def build_program(zero_bias: bool, unit_ln: bool):
    nc = _BaccOneActTable("TRN2", target_bir_lowering=False, debug=False)

    Qd = nc.declare_dram_parameter("Q", [NB, S, D], F32, isOutput=False)
    Kd = nc.declare_dram_parameter("K", [NB, S, D], F32, isOutput=False)
    Wd = {}
    for w in ("Wq", "Wk", "Wv", "Wo"):
        Wd[w] = nc.declare_dram_parameter(w, [D, D], F32, isOutput=False)
    Bd = {}
    for v in ("bq", "bk", "bv", "bo", "ln0_g", "ln0_b", "ln1_g", "ln1_b"):
        Bd[v] = nc.declare_dram_parameter(v, [D], F32, isOutput=False)
    Od = nc.declare_dram_parameter("out", [NB, S, D], F32, isOutput=True)

    with tile.TileContext(nc) as tc:
        _build(nc, tc, Qd, Kd, Wd, Bd, Od, zero_bias, unit_ln)
    nc.compile()
    return nc


def _build(nc, tc, Qd, Kd, Wd, Bd, Od, zero_bias, unit_ln):
    from contextlib import ExitStack

    ctx = ExitStack()
    with ctx:
        const = ctx.enter_context(tc.tile_pool(name="const", bufs=1))
        stage = ctx.enter_context(tc.tile_pool(name="stage", bufs=2))
        loadp = ctx.enter_context(tc.tile_pool(name="loadp", bufs=4))
        n16p = ctx.enter_context(tc.tile_pool(name="n16p", bufs=5))
        t16p = ctx.enter_context(tc.tile_pool(name="t16p", bufs=12))
        projp = ctx.enter_context(tc.tile_pool(name="projp", bufs=17))
        vaugp = ctx.enter_context(tc.tile_pool(name="vaugp", bufs=9))
        qnatp = ctx.enter_context(tc.tile_pool(name="qnatp", bufs=5))
        expp = ctx.enter_context(tc.tile_pool(name="expp", bufs=2))
        atp = ctx.enter_context(tc.tile_pool(name="atp", bufs=5))
        rnp = ctx.enter_context(tc.tile_pool(name="rnp", bufs=3))
        sump = ctx.enter_context(tc.tile_pool(name="sump", bufs=2))
        ohp = ctx.enter_context(tc.tile_pool(name="ohp", bufs=4))
        ln0p = ctx.enter_context(tc.tile_pool(name="ln0p", bufs=4))
        lntp = ctx.enter_context(tc.tile_pool(name="lntp", bufs=3))
        p2p = ctx.enter_context(tc.tile_pool(name="p2p", bufs=5))
        outp = ctx.enter_context(tc.tile_pool(name="outp", bufs=1))
        statp = ctx.enter_context(tc.tile_pool(name="statp", bufs=10))

        # PSUM (8 banks): scores pairs 2x[2 banks], attn-out pair 1x[2],
        # misc (proj/fc/transpose-pairs) 2x[1].
        scp = ctx.enter_context(tc.tile_pool(name="scp", bufs=2, space="PSUM"))
        pop = ctx.enter_context(tc.tile_pool(name="pop", bufs=1, space="PSUM"))
        mp = ctx.enter_context(tc.tile_pool(name="mp", bufs=2, space="PSUM"))

        # ---- one-time constants ----
        I128b = const.tile([P, P], BF16)
        make_identity(nc, I128b)
        I8f = const.tile([H, H], F32)
        make_identity(nc, I8f)
        epsT = const.tile([P, 1], F32)
        nc.vector.memset(epsT[:], EPS)

        W16 = {}
        WST = {}

        def emit_weight_dma(w):
            # split DMA in halves so the first dj blocks are usable before
            # the full matrix has landed
            st = loadp.tile([P, NDB, D], F32, tag="wld", name="wld", bufs=2)
            src = Wd[w].ap().rearrange("(o p) n -> p o n", p=P)
            for hh in range(2):
                nc.sync.dma_start(st[:, 2 * hh:2 * hh + 2, :],
                                  src[:, 2 * hh:2 * hh + 2, :])
            WST[w] = st

        def emit_weight_cast(w, eng):
            W16[w] = const.tile([P, NDB, D], BF16, tag=f"w16_{w}", name=f"w16_{w}")
            st = WST[w]
            for hh in range(2):
                sl = (slice(None), slice(2 * hh, 2 * hh + 2), slice(None))
                if eng == "act":
                    nc.scalar.activation(W16[w][sl], st[sl], AF.Copy)
                elif eng == "vec":
                    nc.vector.tensor_copy(W16[w][sl], st[sl])
                else:
                    nc.gpsimd.tensor_copy(W16[w][sl], st[sl])

        if not zero_bias:
            bqT = const.tile([P, NDB], F32, tag="bqT")
            nc.sync.dma_start(bqT[:], Bd["bq"].ap().rearrange("(o p) -> p o", p=P))
            bkT = const.tile([P, NDB], F32, tag="bkT")
            nc.sync.dma_start(bkT[:], Bd["bk"].ap().rearrange("(o p) -> p o", p=P))
            bc = {}
            for v in ("bv", "bo"):
                st = stage.tile([1, D], F32, tag="vstage")
                nc.sync.dma_start(st[:], Bd[v].ap()[None, :])
                bc[v] = const.tile([P, D], F32, tag=f"bc_{v}", name=f"bc_{v}")
                nc.gpsimd.partition_broadcast(bc[v][:], st[:])
            bv_bc, bo_bc = bc["bv"], bc["bo"]
        if not unit_ln:
            gbc = {}
            for v in ("ln0_g", "ln0_b", "ln1_g", "ln1_b"):
                st = stage.tile([1, D], F32, tag="vstage")
                nc.sync.dma_start(st[:], Bd[v].ap()[None, :])
                gbc[v] = const.tile([P, D], F32, tag=f"bc_{v}", name=f"bc_{v}")
                nc.gpsimd.partition_broadcast(gbc[v][:], st[:])

        # ---- per-batch state ----
        N16 = [{} for _ in range(NB)]       # name -> [128, NBLK, D] bf16
        T16S = [{} for _ in range(NB)]      # name -> [4 tiles d-major]
        PROJ = [{} for _ in range(NB)]      # "qT"/"kT" -> [4 tiles]
        VAUG = [None] * NB
        QNAT = [None] * NB                  # [4 AP views, natural q]
        AT = [[None] * H for _ in range(NB)]
        STG = [None] * NB                   # [H, S] bf16 sums stage
        RN = [None] * NB                    # [P, NBLK, H] f32 reciprocal sums
        OH = [None] * NB                    # [4 AP views, natural oh]
        LN0 = [None] * NB
        LNT = [None] * NB
        PRE2 = [None] * NB

        LDS = [{} for _ in range(NB)]
        RNPEND = [None] * NB

        def emit_load_dma(b, name):
            dram = Qd if name == "Q" else Kd
            ld = loadp.tile([P, NBLK, D], F32, tag="ld", name="ld")
            nc.sync.dma_start(
                ld[:], dram[b].rearrange("(si p) d -> p si d", p=P)
            )
            LDS[b][name] = ld

        def emit_load_cast(b, name):
            n16 = n16p.tile([P, NBLK, D], BF16, tag="n16", name="n16")
            if b == 0:
                # startup: per-dj casts on fast engines so A-groups start
                # as each feature block becomes available
                for dj in range(NDB):
                    sl = (slice(None), slice(None), slice(dj * P, (dj + 1) * P))
                    if name == "Q":
                        nc.scalar.activation(n16[sl], LDS[b][name][sl], AF.Copy)
                    else:
                        nc.vector.tensor_copy(n16[sl], LDS[b][name][sl])
            elif name == "Q":
                nc.scalar.activation(n16[:], LDS[b][name][:], AF.Copy)
            else:
                nc.vector.tensor_copy(n16[:], LDS[b][name][:])
            N16[b][name] = n16

        def ln_stats_si(src):
            mv = statp.tile([P, 2], F32, tag="mv1", name="mv1", bufs=6)
            st6 = statp.tile([P, 6], F32, tag="st6", name="st6")
            nc.vector.bn_stats(st6[:], src[:])
            nc.vector.bn_aggr(mv[:], st6[:])
            lnv = statp.tile([P, 1], F32, tag="lnv1", name="lnv1", bufs=6)
            nc.scalar.activation(lnv[:], mv[:, 1:2], AF.Ln, bias=epsT[:])
            istd = statp.tile([P, 1], F32, tag="istd1", name="istd1", bufs=6)
            nc.scalar.activation(istd[:], lnv[:], AF.Exp, scale=-0.5)
            return mv, istd

        def ln_apply_si(dst, src, mv, istd, g_bc, b_bc):
            if g_bc is None:
                nc.vector.tensor_scalar(
                    dst, src[:], mv[:, 0:1], istd[:], OP.subtract, OP.mult,
                )
            else:
                t = statp.tile([P, D], F32, tag="lntmp", name="lntmp")
                nc.vector.tensor_scalar(
                    t[:], src[:], mv[:, 0:1], istd[:], OP.subtract, OP.mult,
                )
                t2 = statp.tile([P, D], F32, tag="lntmp2", name="lntmp2")
                nc.vector.tensor_tensor(t2[:], t[:], g_bc[:], OP.mult)
                nc.vector.tensor_tensor(dst, t2[:], b_bc[:], OP.add)

        def ln_stats(srcs):
            """srcs: list of NBLK [128, 512] tiles -> (mv4, istd4)."""
            mv4 = statp.tile([P, NBLK, 2], F32, tag="mv4", name="mv4")
            for si in range(NBLK):
                st6 = statp.tile([P, 6], F32, tag="st6", name="st6")
                nc.vector.bn_stats(st6[:], srcs[si][:])
                nc.vector.bn_aggr(mv4[:, si, :], st6[:])
            lnv = statp.tile([P, NBLK], F32, tag="lnv", name="lnv")
            nc.scalar.activation(lnv[:], mv4[:, :, 1], AF.Ln, bias=epsT[:])
            istd4 = statp.tile([P, NBLK], F32, tag="istd4", name="istd4")
            nc.scalar.activation(istd4[:], lnv[:], AF.Exp, scale=-0.5)
            return mv4, istd4

        def ln_apply_into(dst, src, mv4, istd4, si, g_bc, b_bc):
            if g_bc is None:
                nc.vector.tensor_scalar(
                    dst, src[:], mv4[:, si, 0:1], istd4[:, si:si + 1],
                    OP.subtract, OP.mult,
                )
            else:
                t = statp.tile([P, D], F32, tag="lntmp", name="lntmp")
                nc.vector.tensor_scalar(
                    t[:], src[:], mv4[:, si, 0:1], istd4[:, si:si + 1],
                    OP.subtract, OP.mult,
                )
                t2 = statp.tile([P, D], F32, tag="lntmp2", name="lntmp2")
                nc.vector.tensor_tensor(t2[:], t[:], g_bc[:], OP.mult)
                nc.vector.tensor_tensor(dst, t2[:], b_bc[:], OP.add)

        def emit_A_group(b, g):
            # batch 0 (startup): one dj block per psum tile, DVE drain
            name, dj = ("Q", g) if g < NDB else ("K", g - NDB)
            n16 = N16[b][name]
            ps = mp.tile([P, S], BF16, tag="mp", name="psA")
            for si in range(NBLK):
                nc.tensor.transpose(
                    ps[:, si * P:(si + 1) * P],
                    n16[:, si, dj * P:(dj + 1) * P],
                    I128b[:],
                )
            t16 = t16p.tile([P, S], BF16, tag="t16", name="t16", bufs=8)
            nc.vector.tensor_copy(t16[:], ps[:])
            T16S[b].setdefault(name, []).append(t16)

        def emit_A_pair(b, name, dp):
            # steady state: two dj blocks share one [128, 2, 512] psum tile
            # and one drain
            n16 = N16[b][name]
            ps = mp.tile([P, 2, S], BF16, tag="mp", name="psA2")
            for u in range(2):
                dj = 2 * dp + u
                for si in range(NBLK):
                    nc.tensor.transpose(
                        ps[:, u, si * P:(si + 1) * P],
                        n16[:, si, dj * P:(dj + 1) * P],
                        I128b[:],
                    )
            t16 = t16p.tile([P, 2, S], BF16, tag="t16b", name="t16b", bufs=5)
            nc.vector.tensor_copy(t16[:], ps[:])
            lst = T16S[b].setdefault(name, [])
            lst.append(t16[:, 0, :])
            lst.append(t16[:, 1, :])

        PSB = [{} for _ in range(NB)]

        def emit_B_half(b, g, half):
            # B chains split in dj-halves: the open psum tile is held only
            # across ADJACENT filler pieces (no other mp alloc between), so
            # no single filler burst exceeds ~2 matmuls of PE time
            QT16 = T16S[b].get("Q")
            KT16 = T16S[b].get("K")
            djs = (0, 1) if half == 0 else (2, 3)
            if g < 8:  # qT (g 0-3) / kT (g 4-7)
                wname = "Wq" if g < NDB else "Wk"
                bT = None if zero_bias else (bqT if g < NDB else bkT)
                src = QT16 if g < NDB else KT16
                vi = g % NDB
                if half == 0:
                    ps = mp.tile([P, S], F32, tag="mp", name="psB")
                    PSB[b][g] = ps
                else:
                    ps = PSB[b].pop(g)
                for dj in djs:
                    nc.tensor.matmul(
                        ps[:],
                        W16[wname][:, dj, vi * P:(vi + 1) * P],
                        src[dj][:],
                        start=(dj == 0),
                        stop=(dj == NDB - 1),
                    )
                if half == 0:
                    return
                t = projp.tile([P, S], BF16, tag="projT", name="projT")
                # qT and kT drains on ACT (DVE is the busier engine)
                if bT is None:
                    nc.scalar.activation(t[:], ps[:], AF.Copy)
                else:
                    nc.scalar.activation(t[:], ps[:], AF.Identity,
                                         bias=bT[:, vi:vi + 1])
                PROJ[b].setdefault("qT" if g < NDB else "kT", []).append(t)
            else:      # v groups (g 8-11)
                si = g - 8
                if half == 0:
                    ps = mp.tile([P, S], F32, tag="mp", name="psV")
                    PSB[b][g] = ps
                else:
                    ps = PSB[b].pop(g)
                for dj in djs:
                    nc.tensor.matmul(
                        ps[:],
                        KT16[dj][:, si * P:(si + 1) * P],
                        W16["Wv"][:, dj, :],
                        start=(dj == 0),
                        stop=(dj == NDB - 1),
                    )
                if half == 0:
                    return
                if VAUG[b] is None:
                    VAUG[b] = []
                va = vaugp.tile([P, H, DH + 1], BF16, tag="vaug", name="vaug")
                nc.vector.memset(va[:, :, DH:DH + 1], 1.0)
                if zero_bias:
                    nc.vector.tensor_copy(
                        va[:, :, 0:DH], ps.rearrange("p (h d) -> p h d", h=H)
                    )
                else:
                    nc.vector.tensor_tensor(
                        va[:, :, 0:DH],
                        ps.rearrange("p (h d) -> p h d", h=H),
                        bv_bc.rearrange("p (h d) -> p h d", h=H),
                        OP.add,
                    )
                VAUG[b].append(va)

        def emit_B_group(b, g):
            emit_B_half(b, g, 0)
            emit_B_half(b, g, 1)

        def emit_qnat_pair(b, sp):
            # si blocks (2sp, 2sp+1) share one psum tile + one drain
            qT16 = PROJ[b]["qT"]
            ps = mp.tile([P, 2, S], BF16, tag="mp", name="psQn")
            for u in range(2):
                si = 2 * sp + u
                for vi in range(NDB):
                    nc.tensor.transpose(
                        ps[:, u, vi * P:(vi + 1) * P],
                        qT16[vi][:, si * P:(si + 1) * P],
                        I128b[:],
                    )
            if QNAT[b] is None:
                QNAT[b] = []
            qn = qnatp.tile([P, 2, S], BF16, tag="qnat", name="qnat")
            nc.vector.tensor_copy(qn[:], ps[:])
            QNAT[b].append(qn[:, 0, :])
            QNAT[b].append(qn[:, 1, :])

        def emit_C_pair(b, hp, filler=None):
            # heads (2hp, 2hp+1) share feature block hp; per ki both heads'
            # score matmuls -> one [128, 2, 512] PSUM tile, one exp per ki,
            # then both heads' A@V accumulate in one [128, 2, 512] psum
            # drained once per pair.  The PE stream is software-pipelined:
            # scores(ki+1) issue BEFORE A@V(ki) so the PE never sits queued
            # behind exp(ki); `filler` emits one interleave piece per
            # ki-step to keep every engine's queue fed.
            qT16, kT16 = PROJ[b]["qT"], PROJ[b]["kT"]
            if STG[b] is None:
                STG[b] = sump.tile([H, S], BF16, tag="stg", name="stg")
            vi = hp
            ea = expp.tile([P, 2, NBLK, S], BF16, tag="expA", name="expA")
            pos = pop.tile([P, 2, S], F32, tag="po", name="po")

            def emit_scores(ki):
                ps = scp.tile([P, 2, S], F32, tag="scp", name="scp")
                for u in range(2):
                    hof = u * DH
                    nc.tensor.matmul(
                        ps[:, u, :],
                        kT16[vi][hof:hof + DH, ki * P:(ki + 1) * P],
                        qT16[vi][hof:hof + DH, :],
                        start=True,
                        stop=True,
                    )
                return ps

            def emit_av(ki):
                for u in range(2):
                    h = 2 * hp + u
                    nc.tensor.matmul(
                        pos[0:DH + 1, u, :],
                        VAUG[b][ki][:, h, :],
                        ea[:, u, ki, :],
                        start=(ki == 0),
                        stop=(ki == NBLK - 1),
                    )

            for ki in range(NBLK):
                ps = emit_scores(ki)
                if ki >= 2:
                    emit_av(ki - 2)
                nc.scalar.activation(
                    ea[:, :, ki, :], ps[:], AF.Exp, scale=SC,
                )
                if filler is not None:
                    filler()
            emit_av(NBLK - 2)
            emit_av(NBLK - 1)
            at = atp.tile([DH + 1, 2, S], BF16, tag="at", name="at")
            if b == NB - 1 and hp == H // 2 - 1:
                # final pair: ACT is idle once its exps end, and the DVE
                # queue is the tail spine — drain there instead
                nc.scalar.activation(at[:], pos[0:DH + 1, :, :], AF.Copy)
            else:
                nc.vector.tensor_copy(at[:], pos[0:DH + 1, :, :])
            for u in range(2):
                h = 2 * hp + u
                # sums row -> stage tile (SBUF->SBUF DMA; no DRAM trip)
                nc.sync.dma_start(STG[b][h:h + 1, :], at[DH:DH + 1, u, :])
                AT[b][h] = at[:, u, :]

        PSD = [{} for _ in range(NB)]

        def emit_D_half(b, sp, half):
            # si blocks (2sp, 2sp+1): 16 transposes -> one [128, 2, 512]
            # psum -> one rn-mult drain -> two residual adds (gpsimd);
            # split in two adjacent filler pieces (8 transposes each)
            if half == 0:
                pa = mp.tile([P, 2, S], BF16, tag="mp", name="psD")
                PSD[b][sp] = pa
            else:
                pa = PSD[b].pop(sp)
            u = half
            si = 2 * sp + u
            for h in range(H):
                nc.tensor.transpose(
                    pa[:, u, h * DH:(h + 1) * DH],
                    AT[b][h][0:DH, si * P:(si + 1) * P],
                    I128b[0:DH, 0:DH],
                )
            if half == 0:
                return
            rn = RN[b]
            if OH[b] is None:
                OH[b] = []
            o = ohp.tile([P, 2, S], BF16, tag="oh", name="oh", bufs=3)
            nc.vector.tensor_tensor(
                o.rearrange("p u (h d) -> p u h d", h=H),
                pa.rearrange("p u (h d) -> p u h d", h=H),
                rn[:, 2 * sp:2 * sp + 2, :, None].to_broadcast((P, 2, H, DH)),
                OP.mult,
            )
            for u in range(2):
                si = 2 * sp + u
                nc.gpsimd.tensor_tensor(o[:, u, :], o[:, u, :],
                                        QNAT[b][si], OP.add)
                OH[b].append(o[:, u, :])

        def emit_rn(b):
            # stage [H, S] -> per-si PE transpose -> [P, si, H] psum -> 1/x
            ps = mp.tile([P, NBLK, H], BF16, tag="mp", name="psRn")
            for si in range(NBLK):
                nc.tensor.transpose(
                    ps[:, si, :],
                    STG[b][:, si * P:(si + 1) * P],
                    I128b[0:H, 0:H],
                )
            rn = rnp.tile([P, NBLK, H], F32, tag="rn", name="rn")
            nc.vector.reciprocal(rn[:], ps[:])
            RN[b] = rn

        def emit_D_pair(b, sp):
            emit_D_half(b, sp, 0)
            emit_D_half(b, sp, 1)

        def emit_D_group(b, si, last):
            # unpacked per-si variant (last batch: lower tail latency)
            rn = RN[b]
            pa = mp.tile([P, S], BF16, tag="mp", name="psD1")
            for h in range(H):
                nc.tensor.transpose(
                    pa[:, h * DH:(h + 1) * DH],
                    AT[b][h][0:DH, si * P:(si + 1) * P],
                    I128b[0:DH, 0:DH],
                )
            if OH[b] is None:
                OH[b] = []
            o = ohp.tile([P, S], BF16, tag="oh1", name="oh1", bufs=4)
            nc.vector.tensor_tensor(
                o.rearrange("p (h d) -> p h d", h=H),
                pa.rearrange("p (h d) -> p h d", h=H),
                rn[:, si, :, None].to_broadcast((P, H, DH)),
                OP.mult,
            )
            if last:
                nc.vector.tensor_tensor(o[:], o[:], QNAT[b][si], OP.add)
            else:
                nc.gpsimd.tensor_tensor(o[:], o[:], QNAT[b][si], OP.add)
            OH[b].append(o[:])

        MV4 = [None] * NB

        def emit_E_stats(b, si):
            if MV4[b] is None:
                MV4[b] = statp.tile([P, NBLK, 2], F32, tag="mv4", name="mv4")
            st6 = statp.tile([P, 6], F32, tag="st6", name="st6")
            nc.vector.bn_stats(st6[:], OH[b][si])
            nc.vector.bn_aggr(MV4[b][:, si, :], st6[:])

        def emit_E_fin(b):
            g0 = None if unit_ln else gbc["ln0_g"]
            b0 = None if unit_ln else gbc["ln0_b"]
            mv4 = MV4[b]
            lnv = statp.tile([P, NBLK], F32, tag="lnv", name="lnv")
            nc.scalar.activation(lnv[:], mv4[:, :, 1], AF.Ln, bias=epsT[:])
            istd4 = statp.tile([P, NBLK], F32, tag="istd4", name="istd4")
            nc.scalar.activation(istd4[:], lnv[:], AF.Exp, scale=-0.5)
            LN0[b] = []
            for si in range(NBLK):
                dst = ln0p.tile([P, D], BF16, tag="ln0", name="ln0")
                ln_apply_into(dst[:], OH[b][si], mv4, istd4, si, g0, b0)
                LN0[b].append(dst)

        def emit_F_lnT_pair(b, vp):
            # vi blocks (2vp, 2vp+1) share one psum tile + one drain
            ps = mp.tile([P, 2, S], BF16, tag="mp", name="psF")
            for u in range(2):
                vi = 2 * vp + u
                for si in range(NBLK):
                    nc.tensor.transpose(
                        ps[:, u, si * P:(si + 1) * P],
                        LN0[b][si][:, vi * P:(vi + 1) * P],
                        I128b[:],
                    )
            if LNT[b] is None:
                LNT[b] = []
            t = lntp.tile([P, 2, S], BF16, tag="lnT", name="lnT")
            nc.vector.tensor_copy(t[:], ps[:])
            LNT[b].append(t[:, 0, :])
            LNT[b].append(t[:, 1, :])

        PSF = [{} for _ in range(NB)]

        def emit_F_fc_half(b, si, half):
            if half == 0:
                ps = mp.tile([P, S], F32, tag="mp", name="psFc")
                PSF[b][si] = ps
            else:
                ps = PSF[b].pop(si)
            for dj in ((0, 1) if half == 0 else (2, 3)):
                nc.tensor.matmul(
                    ps[:],
                    LNT[b][dj][:, si * P:(si + 1) * P],
                    W16["Wo"][:, dj, :],
                    start=(dj == 0),
                    stop=(dj == NDB - 1),
                )
            if half == 0:
                return
            if PRE2[b] is None:
                PRE2[b] = []
            p2 = p2p.tile([P, D], BF16, tag="pre2", name="pre2")
            if zero_bias:
                # p2 = relu(fc) + ln0 fused: (ps max 0) + ln0
                nc.vector.scalar_tensor_tensor(
                    p2[:], ps[:], 0.0, LN0[b][si][:], OP.max, OP.add
                )
            else:
                tmp = statp.tile([P, D], F32, tag="fcb", name="fcb")
                nc.vector.tensor_tensor(tmp[:], ps[:], bo_bc[:], OP.add)
                rl = statp.tile([P, D], BF16, tag="relu", name="relu")
                nc.scalar.activation(rl[:], tmp[:], AF.Relu)
                nc.vector.tensor_tensor(p2[:], rl[:], LN0[b][si][:], OP.add)
            PRE2[b].append(p2)

        def emit_F_fc(b, si):
            emit_F_fc_half(b, si, 0)
            emit_F_fc_half(b, si, 1)

        MV4E = [None] * NB

        def emit_E2_stats(b, si):
            # per-si stats run as C-phase fillers so the batch-boundary
            # E2 leaves only Ln/Exp + applies + DMA
            if MV4E[b] is None:
                MV4E[b] = statp.tile([P, NBLK, 2], F32, tag="mv4e",
                                     name="mv4e", bufs=2)
            st6 = statp.tile([P, 6], F32, tag="st6", name="st6")
            nc.vector.bn_stats(st6[:], PRE2[b][si][:])
            nc.vector.bn_aggr(MV4E[b][:, si, :], st6[:])

        def emit_E2(b):
            g1 = None if unit_ln else gbc["ln1_g"]
            b1 = None if unit_ln else gbc["ln1_b"]
            if b == NB - 1:
                for si in range(NBLK):
                    mv, istd = ln_stats_si(PRE2[b][si])
                    of1 = outp.tile([P, D], F32, tag="outf1", name="outf1",
                                    bufs=2)
                    ln_apply_si(of1[:], PRE2[b][si], mv, istd, g1, b1)
                    nc.sync.dma_start(Od[b, si * P:(si + 1) * P, :], of1[:])
                return
            mv4b = MV4E[b]
            lnvb = statp.tile([P, NBLK], F32, tag="lnv", name="lnv")
            nc.scalar.activation(lnvb[:], mv4b[:, :, 1], AF.Ln, bias=epsT[:])
            istd4b = statp.tile([P, NBLK], F32, tag="istd4", name="istd4")
            nc.scalar.activation(istd4b[:], lnvb[:], AF.Exp, scale=-0.5)
            of = outp.tile([P, NBLK, D], F32, tag="outf", name="outf")
            if unit_ln and b == NB - 2:
                # this E2's applies land right before the last batch's tail
                # spine on DVE; run them on ACT (idle once the exps end)
                nm4b = statp.tile([P, NBLK], F32, tag="nm4", name="nm4")
                nc.vector.tensor_tensor(nm4b[:], mv4b[:, :, 0], istd4b[:],
                                        OP.mult)
                nc.vector.tensor_scalar(nm4b[:], nm4b[:], -1.0, None, OP.mult)
                for si in range(NBLK):
                    nc.scalar.activation(of[:, si, :], PRE2[b][si][:],
                                         AF.Identity,
                                         bias=nm4b[:, si:si + 1],
                                         scale=istd4b[:, si:si + 1])
            else:
                for si in range(NBLK):
                    ln_apply_into(of[:, si, :], PRE2[b][si], mv4b, istd4b,
                                  si, g1, b1)
            nc.sync.dma_start(
                Od[b].rearrange("(si p) d -> p si d", p=P), of[:]
            )

        # ---- staged emission: 3 batches in flight ----
        def tail_pieces(b):
            th = []
            for sp in range(2):
                th.append(lambda sp=sp: emit_D_pair(b, sp))

            def emit_E(b=b):
                for si in range(NBLK):
                    emit_E_stats(b, si)
                emit_E_fin(b)

            th.append(emit_E)
            for vp in range(2):
                th.append(lambda vp=vp: emit_F_lnT_pair(b, vp))
            for si in range(NBLK):
                th.append(lambda si=si: emit_F_fc(b, si))
                th.append(lambda si=si: emit_E2_stats(b, si))
            return th  # 13 pieces; E2 finalize emitted separately

        def prep_pieces(nb):
            th = []
            for name in ("Q", "K"):
                for dp in range(2):
                    th.append(lambda name=name, dp=dp: emit_A_pair(nb, name, dp))
            for g in range(12):
                th.append(lambda g=g: emit_B_group(nb, g))
            return th  # 16 pieces; qnat emitted post-pairs

        # prologue: batch-0 path to first PE work.  Emission order is
        # engine-queue order, so per-engine the load casts lead the weight
        # casts and each stage's consumers directly follow its producers.
        emit_load_dma(0, "Q")
        emit_weight_dma("Wq")
        emit_load_dma(0, "K")
        emit_weight_dma("Wk")
        emit_load_cast(0, "Q")        # ACT per-dj
        for g in range(NDB):          # A-groups Q (t16 drains on DVE)
            emit_A_group(0, g)
        emit_weight_cast("Wq", "act")
        emit_load_cast(0, "K")        # DVE per-dj
        # Wv/Wo transfers deferred: Q/Wq/K/Wk keep full HBM bandwidth
        # through the startup-critical window
        emit_weight_dma("Wv")
        emit_weight_dma("Wo")
        for g in range(4):            # B qT chains (drains on ACT)
            emit_B_group(0, g)
        for g in range(NDB, 2 * NDB):  # A-groups K
            emit_A_group(0, g)
        emit_weight_cast("Wk", "vec")
        for g in range(4, 8):         # B kT chains
            emit_B_group(0, g)
        emit_weight_cast("Wv", "vec")
        for g in range(8, 12):        # B v chains -> VAUG
            emit_B_group(0, g)
        emit_weight_cast("Wo", "pool")
        for sp in range(2):
            emit_qnat_pair(0, sp)
        emit_load_dma(1, "Q")
        emit_load_dma(1, "K")

        for b in range(NB):
            nb = b + 1
            tails = tail_pieces(b - 1) if b > 0 else []
            preps = prep_pieces(nb) if nb < NB else []
            # A-pairs first so their PSUM drains lead the DVE queue (keeps
            # the mp ring moving), then D/E, then B chains, then lnT/fc
            inter = preps[:4] + tails[:3] + preps[4:] + tails[3:]
            if b == 0:
                inter = [lambda: emit_load_cast(1, "Q"),
                         lambda: emit_load_cast(1, "K")] + inter
            if b + 2 < NB:
                emit_load_dma(b + 2, "Q")
                emit_load_dma(b + 2, "K")
            # distribute interleave pieces one per ki-step (16 slots across
            # the 4 C-pairs) so mp-ring groups never run back-to-back
            state = {"fi": 0, "emitted": 0, "slot": 0}
            nslots = (H // 2) * NBLK

            def filler():
                state["slot"] += 1
                target = (len(inter) * state["slot"] + nslots - 1) // nslots
                while state["fi"] < len(inter) and state["emitted"] < target:
                    inter[state["fi"]]()
                    state["fi"] += 1
                    state["emitted"] += 1

            for hp in range(H // 2):
                emit_C_pair(b, hp, filler)
                if hp == H // 2 - 1:
                    emit_rn(b)
            while state["fi"] < len(inter):
                inter[state["fi"]]()
                state["fi"] += 1
            if b > 0:
                emit_E2(b - 1)
            if nb < NB:
                for sp in range(2):
                    emit_qnat_pair(nb, sp)
            if b + 2 < NB:
                emit_load_cast(b + 2, "Q")
                emit_load_cast(b + 2, "K")

        # epilogue: last batch tail; interleave stats with per-si D groups,
        # run LN applies on ACT, chain fc/p2/E2 per-si
        b = NB - 1
        if unit_ln:
            emit_D_group(b, 0, True)
            emit_D_group(b, 1, True)
            mv4 = statp.tile([P, NBLK, 2], F32, tag="mv4", name="mv4")

            def tail_stats(si):
                st6 = statp.tile([P, 6], F32, tag="st6", name="st6")
                nc.vector.bn_stats(st6[:], OH[b][si])
                nc.vector.bn_aggr(mv4[:, si, :], st6[:])

            tail_stats(0)
            emit_D_group(b, 2, True)
            tail_stats(1)
            emit_D_group(b, 3, True)
            tail_stats(2)
            tail_stats(3)
            lnv = statp.tile([P, NBLK], F32, tag="lnv", name="lnv")
            nc.scalar.activation(lnv[:], mv4[:, :, 1], AF.Ln, bias=epsT[:])
            istd4 = statp.tile([P, NBLK], F32, tag="istd4", name="istd4")
            nc.scalar.activation(istd4[:], lnv[:], AF.Exp, scale=-0.5)
            nm4 = statp.tile([P, NBLK], F32, tag="nm4", name="nm4")
            nc.vector.tensor_tensor(nm4[:], mv4[:, :, 0], istd4[:], OP.mult)
            nc.vector.tensor_scalar(nm4[:], nm4[:], -1.0, None, OP.mult)
            LN0[b] = []
            for si in range(NBLK):
                # alternate apply engines so the applies window halves and
                # each si's lnT transposes start sooner
                dst = ln0p.tile([P, D], BF16, tag="ln0", name="ln0")
                if si % 2 == 0:
                    nc.scalar.activation(dst[:], OH[b][si], AF.Identity,
                                         bias=nm4[:, si:si + 1],
                                         scale=istd4[:, si:si + 1])
                else:
                    nc.vector.tensor_scalar(
                        dst[:], OH[b][si], mv4[:, si, 0:1],
                        istd4[:, si:si + 1], OP.subtract, OP.mult,
                    )
                LN0[b].append(dst)
            # lnT transposes ordered si-major: each si's transposes chase
            # its LN apply, so the PE stays warm through the applies window
            # (both mp slots held; no other mp alloc until the drains)
            psF = [mp.tile([P, 2, S], BF16, tag="mp", name="psF")
                   for _ in range(2)]
            for si in range(NBLK):
                for vp in range(2):
                    for u in range(2):
                        vi = 2 * vp + u
                        nc.tensor.transpose(
                            psF[vp][:, u, si * P:(si + 1) * P],
                            LN0[b][si][:, vi * P:(vi + 1) * P],
                            I128b[:],
                        )
            LNT[b] = []
            for vp in range(2):
                t = lntp.tile([P, 2, S], BF16, tag="lnT", name="lnT")
                nc.vector.tensor_copy(t[:], psF[vp][:])
                LNT[b].append(t[:, 0, :])
                LNT[b].append(t[:, 1, :])
            for si in range(NBLK):
                emit_F_fc(b, si)
                # E2 per-si immediately after its fc/p2; applies alternate
                # ACT/DVE so consecutive si overlap
                mv, istd = ln_stats_si(PRE2[b][si])
                of1 = outp.tile([P, D], F32, tag="outf1", name="outf1",
                                bufs=2)
                if si % 2 == 0:
                    nm = statp.tile([P, 1], F32, tag="nm1", name="nm1",
                                    bufs=4)
                    nc.vector.tensor_scalar(
                        nm[:], mv[:, 0:1], istd[:], -1.0, OP.mult, OP.mult,
                    )
                    nc.scalar.activation(of1[:], PRE2[b][si][:], AF.Identity,
                                         bias=nm[:], scale=istd[:])
                else:
                    nc.vector.tensor_scalar(
                        of1[:], PRE2[b][si][:], mv[:, 0:1], istd[:],
                        OP.subtract, OP.mult,
                    )
                nc.sync.dma_start(Od[b, si * P:(si + 1) * P, :], of1[:])
        else:
            for th in tail_pieces(NB - 1):
                th()
            emit_E2(NB - 1)


_CACHE = {}


def _get_program(zero_bias: bool, unit_ln: bool):
    key = (zero_bias, unit_ln)
    if key not in _CACHE:
        _CACHE[key] = build_program(zero_bias, unit_ln)
    return _CACHE[key]


def _make_in_maps(inputs):
    Q = np.ascontiguousarray(inputs["Q"], dtype=np.float32)
    K = np.ascontiguousarray(inputs["K"], dtype=np.float32)
    shared = {
        name: np.ascontiguousarray(inputs[name], dtype=np.float32)
        for name in ("Wq", "Wk", "Wv", "Wo", "bq", "bk", "bv", "bo",
                     "ln0_g", "ln0_b", "ln1_g", "ln1_b")
    }
    in_maps = []
    for c in range(NCORES):
        m = dict(shared)
        m["Q"] = Q[c * NB:(c + 1) * NB]
        m["K"] = K[c * NB:(c + 1) * NB]
        in_maps.append(m)
    return in_maps


def run(inputs, trace=False):
    zero_bias = all(
        not np.any(inputs[v]) for v in ("bq", "bk", "bv", "bo")
    )
    unit_ln = (
        np.all(inputs["ln0_g"] == 1.0) and np.all(inputs["ln1_g"] == 1.0)
        and not np.any(inputs["ln0_b"]) and not np.any(inputs["ln1_b"])
    )
    nc = _get_program(zero_bias, unit_ln)
    res = run_bass_kernel_spmd(
        nc, _make_in_maps(inputs), core_ids=list(range(NCORES)), trace=trace
    )
    out = np.concatenate([res.results[c]["out"] for c in range(NCORES)], axis=0)
    return out, res


def kernel(**inputs):
    B, Sq, Dq = inputs["Q"].shape
    assert (B, Sq, Dq) == (NB * NCORES, S, D), (B, Sq, Dq)
    out, _ = run(inputs, trace=False)
    return out
